# revision 27
# baseline (speedup 1.0000x reference)
"""LoRA linear on 8 trn2 NeuronCores.

out = x @ W.T + b + 2.0 * ((x @ A.T) @ B.T)
x [8192, 4096] f32, W [4096, 4096], b [4096], A [16, 4096], B [4096, 16].

Sharding: data-parallel over tokens (8 x 1024 per core).

The dense x@W.T runs in fp8(e4m3) with MatmulPerfMode.DoubleRow (k=256 per
matmul pass; HW peak 157 TF/s = 1 cycle per output row at 2.4 GHz). Per-core
tensor-engine floor is ~221us for the mains, so the kernel is organized to
keep the PE at 100% from first x8 arrival to the last o-tile:

 - LoRA path: xa = x@A.T (rank 16) uses only 16 of 128 PE columns, so it is
   computed as 4 col-tiled CONCURRENT accumulation chains (tile_position=
   (0,32j)) into one PSUM bank; 4 partition slices are then summed on DVE.
   ~4x faster than a serial rank-16 chain.
 - The first CUT o-tiles drain the mains-only PSUM to SBUF (bf16) without
   waiting for xa; their rank-17 LoRA close runs later ((o*INV_S)+lora via
   one scalar_tensor_tensor). This removes the xa -> mains PSUM-recycling
   deadline that stalled the PE for ~28us in the previous version. O-tiles
   >= CUT fold the close into the same PSUM accumulation (k=17 extra pass).
 - y is stored bf16 (halves write traffic; ~0.1% rel err, gate is 2e-2).
 - W prefetch 10 tiles deep on the sync HWDGE queue; x8 split across both
   queues so the mains start ~3us in; xb (for xa) trickles behind on the
   scalar queue.
"""

import os
import sys
import types

for _p in ("/opt/trn_rl_repo", "/root/.axon_site/_ro/trn_rl_repo"):
    if os.path.isdir(_p) and _p not in sys.path:
        sys.path.append(_p)

import numpy as np
import ml_dtypes


def _ensure_axon_hooks():
    """bass_utils trace=True needs antenv.axon_hooks; some images lack it."""
    try:
        import antenv.axon_hooks  # noqa: F401
        return
    except Exception:
        pass
    mod = types.ModuleType("antenv.axon_hooks")
    mod._hook = None

    def set_axon_ntff_profile_hook(hook):
        mod._hook = hook

    def get_axon_ntff_profile_hook():
        if mod._hook is None:
            try:
                from trn_agent_boot.trn_boot import _ntff_profile_via_ctypes

                mod._hook = _ntff_profile_via_ctypes("/opt/axon/libaxon_pjrt.so")
            except Exception:
                return None
        return mod._hook

    mod.set_axon_ntff_profile_hook = set_axon_ntff_profile_hook
    mod.get_axon_ntff_profile_hook = get_axon_ntff_profile_hook
    try:
        import antenv

        antenv.axon_hooks = mod
    except Exception:
        pass
    sys.modules["antenv.axon_hooks"] = mod


_ensure_axon_hooks()

import concourse.bass as bass
import concourse.bass_utils as bass_utils
import concourse.mybir as mybir
import concourse.tile as tile_mod
from concourse.bass_utils import run_bass_kernel_spmd
from concourse.tile import add_dep_helper

# no fish bucket inside the container; keep artifacts local
bass_utils.upload_artifacts = lambda tmpdir: tmpdir


# ---------------------------------------------------------------------------
# Workarounds for this walrus build: it rejects any instruction that carries
# more than one semaphore wait ("Too many sync wait commands").  (a) replace
# the TileContext tail drain (stacks the whole global clock on one Drain),
# (b) split every multi-wait instruction in the serialized BIR into
# single-wait NoOps placed immediately before it (waits are AND conditions,
# so sequential single waits on the same engine are equivalent).
# ---------------------------------------------------------------------------


def _install_patches():
    from concourse.vector_clock import ScopedClock

    if not getattr(tile_mod.TileContext, "_drain_patch_installed", False):

        def _drain_and_barrier(self, tick_clock, wait_clock):
            nop_inst = self.nc.sync.nop(nofuse=True, hint="pre_drain_waits")
            wait_clock.add_sem_waits(
                nop_inst.ins, ScopedClock({None: tick_clock.global_clock})
            )
            si = nop_inst.ins.sync_info
            if si is not None and si.on_wait and len(si.on_wait) > 1:
                waits = list(si.on_wait)
                si.on_wait = waits[:1]
                for w in waits[1:]:
                    n2 = self.nc.sync.nop(nofuse=True, hint="pre_drain_waits")
                    n2.ins.sync_info = mybir.SyncInfo(on_wait=[w], on_update=[])
            self.nc.sync.drain()
            self.nc.all_engine_barrier()
            assert self.sems is not None
            popped = self.nc._tile_sem_poison_stack.pop()
            assert popped is self._sem_poison
            self.nc.clear_and_free_semaphores(list(self.sems.allocated().values()))
            self.nc.all_engine_barrier()

        tile_mod.TileContext._drain_and_barrier = _drain_and_barrier
        tile_mod.TileContext._drain_patch_installed = True

    if not getattr(bass.Bass, "_wait_split_installed", False):
        import json

        def _split_waits_json(raw):
            d = json.loads(raw)
            n = 0
            for f in d.get("functions", []):
                for b in f.get("blocks", []):
                    out = []
                    for inst in b.get("instructions", []):
                        si = inst.get("sync_info")
                        if si:
                            waits = si.get("on_wait") or []
                            if len(waits) > 1:
                                for w in waits[:-1]:
                                    n += 1
                                    nop = {
                                        "engine": inst["engine"],
                                        "ins": [],
                                        "outs": [],
                                        "name": f"wsplit-{n}",
                                        "opcode": "NoOp",
                                        "sync_info": {
                                            "on_update": [],
                                            "on_wait": [w],
                                        },
                                        "text_hint": "wsplit",
                                    }
                                    if "debug" in inst:
                                        nop["debug"] = inst["debug"]
                                    out.append(nop)
                                si["on_wait"] = [waits[-1]]
                        out.append(inst)
                    b["instructions"] = out
            return json.dumps(d).encode()

        def to_json_bytes(self):
            return _split_waits_json(mybir.module_to_json_bytes(self.m))

        bass.Bass.to_json_bytes = to_json_bytes
        bass.Bass._wait_split_installed = True


_install_patches()

# ---------------------------------------------------------------------------

N_CORES = 8
NTOK = 8192
K = 4096
O = 4096
R = 16
SCALING = 2.0

T = NTOK // N_CORES      # 1024 tokens per core
KC2 = K // 256           # 16 k-pairs (DoubleRow: 2 planes of 128 per matmul)
KC = K // 128            # 32 k-chunks for the xa (LoRA) path
OT = O // 128            # 32 o-tiles
TT = T // 512            # 2 token tiles of 512

SX = 8.0                 # fp8 scale for x
SW = 256.0               # fp8 scale for W
INV_S = 1.0 / (SX * SW)

XG = 16                  # x8 load split (1 k-pair per part)
XBG = 8                  # xb load split (4 k-chunks per part)
LAG = 3                  # mains psum depth: 2*LAG banks
CUT = 16                 # o-tiles 0..CUT-1 use the decoupled (late) close
XA_R0 = 7                # xa round r is emitted before mains(XA_R0 + r)
LC0 = 16                 # late close q is emitted before mains(LC0 + q)
WBUFS = 8                # W prefetch depth

F32 = mybir.dt.float32
BF16 = mybir.dt.bfloat16
F8 = mybir.dt.float8e4
FP8_NP = ml_dtypes.float8_e4m3
ADD = mybir.AluOpType.add
MULT = mybir.AluOpType.mult

LAST_RESULT = None  # test harness reads exec_time_ns off this


def _build_kernel():
    nc = bass.Bass("TRN2", num_devices=N_CORES)

    x8_in = nc.declare_dram_parameter("x8", [128, KC2, 2, T], F8, isOutput=False)
    xb_in = nc.declare_dram_parameter("xb", [128, KC, T], BF16, isOutput=False)
    w8_in = nc.declare_dram_parameter("w8", [OT, 128, KC2, 2, 128], F8, isOutput=False)
    at_in = nc.declare_dram_parameter("at", [128, KC, R], BF16, isOutput=False)
    # rank-17 stationaries: rows 0..15 = 2*SX*SW*B.T, row 16 = SX*SW*b,
    # DUPLICATED at partition offset 32 so the t=0/t=1 closes can run as
    # row-tiled CONCURRENT matmuls (tile_position=(0,0)/(32,0)).
    # btb: used by the folded close (drain then scales by INV_S).
    # btbl = btb*INV_S: used by the late close (mains drained unscaled).
    btb_in = nc.declare_dram_parameter("btb", [32 + R + 1, O], BF16, isOutput=False)
    btbl_in = nc.declare_dram_parameter("btbl", [32 + R + 1, O], BF16, isOutput=False)
    y_out = nc.declare_dram_parameter("y", [OT, 128, T], BF16, isOutput=True)

    with tile_mod.TileContext(nc) as tc:
        with (
            tc.tile_pool(name="xp", bufs=1) as xp,
            tc.tile_pool(name="cp", bufs=1) as cp,
            tc.tile_pool(name="wp", bufs=WBUFS) as wp,
            tc.tile_pool(name="obp", bufs=CUT) as obp,
            tc.tile_pool(name="yp", bufs=6) as yp,
            tc.tile_pool(name="psp", bufs=2 * LAG, space="PSUM") as psp,
            tc.tile_pool(name="psx", bufs=2, space="PSUM") as psx,
        ):
            # --- input DMAs.  Queue layout (both FIFO; SDMA engines round-
            # robin between them): sync carries W only; scalar carries x8
            # FIRST (the mains' critical path), then the consts and xb.
            # Few-partition consts (btb: 17 partitions) go LAST among the
            # early loads -- ahead of x8 they serialize the queue head.
            w_tiles = {}

            def issue_w(ot, engine):
                w_sb = wp.tile([128, KC2, 2, 128], F8, tag="w", name=f"w{ot}")
                engine.dma_start(w_sb[:], w8_in[ot])
                w_tiles[ot] = w_sb

            issue_w(0, nc.sync)
            issue_w(1, nc.sync)

            # x8 parts split across both queues: evens head the scalar queue,
            # odds go on sync behind W0/W1 -- x8 streams at full node rate
            x8_parts = [None] * XG
            cpp8 = KC2 // XG
            for g in range(XG):
                t8 = xp.tile([128, cpp8, 2, T], F8, tag=f"x8_{g}")
                eng = nc.scalar if g % 2 == 0 else nc.sync
                eng.dma_start(t8[:], x8_in[:, g * cpp8:(g + 1) * cpp8, :, :])
                x8_parts[g] = t8

            for ot in range(2, WBUFS):
                issue_w(ot, nc.sync)

            at_sb = cp.tile([128, KC, R], BF16)
            nc.scalar.dma_start(at_sb[:], at_in[:])
            btb_sb = cp.tile([32 + R + 1, O], BF16)
            nc.scalar.dma_start(btb_sb[:], btb_in[:], max_dma_last_dim=1024)
            btbl_sb = cp.tile([32 + R + 1, O], BF16)
            nc.scalar.dma_start(btbl_sb[:], btbl_in[:], max_dma_last_dim=1024)

            xb_parts = []
            cppb = KC // XBG
            for g in range(XBG):
                tb = xp.tile([128, cppb, T], BF16, tag=f"xb_{g}")
                nc.scalar.dma_start(tb[:], xb_in[:, g * cppb:(g + 1) * cppb, :])
                xb_parts.append(tb)

            # xa_ext rows 0..15 = x@A.T (written by the combines), row 16 = 1
            # (bias row of the rank-17 close); rows 32..48 = duplicate for
            # the row-tiled t=1 close.  memset everything to 1.0 (covers both
            # bias rows); combines/dup-copy overwrite the xa rows later.
            xa_sb = cp.tile([32 + R + 1, T], BF16)
            nc.vector.memset(xa_sb[0:32 + R + 1, :], 1.0)
            xcmb = [cp.tile([R, 512], F32, name=f"xcmb{t}") for t in range(TT)]

            def x8_sl(c, t):  # [128, 2, 512] fp8 moving chunk
                g, cl = divmod(c, cpp8)
                return x8_parts[g][:, cl, :, t * 512:(t + 1) * 512]

            def xb_sl(c, t):  # [128, 512] bf16 moving chunk
                g, cl = divmod(c, cppb)
                return xb_parts[g][:, cl, t * 512:(t + 1) * 512]

            # --- xa: 4 concurrent col-tiled rank-16 chains; slice j (PE cols
            # 32j..) accumulates chunks c = 4r+j over rounds r; one bank per
            # token half.
            xa_ps = [
                psx.tile([128, 512], F32, tag="px", name=f"xa_ps{t}")
                for t in range(TT)
            ]

            def emit_xa_round(r):
                first = True
                for t in range(TT):
                    for j in range(4):
                        c = 4 * r + j
                        mm = nc.tensor.matmul(
                            xa_ps[t][32 * j:32 * j + R, :],
                            at_sb[:, c, :],
                            xb_sl(c, t),
                            start=(r == 0),
                            stop=(r == KC // 4 - 1),
                            tile_position=(0, 32 * j),
                        )
                        if first and last_mains_mm[0] is not None:
                            # keep the round behind the preceding mains tile so
                            # the scheduler cannot hoist it ahead of ready
                            # mains work (it would stall the PE on the xb DMA)
                            add_dep_helper(
                                mm.ins, last_mains_mm[0], reason="xa after mains"
                            )
                            first = False

            def emit_xa_combine():
                # cascade adds (DVE reads at most one PSUM input per op)
                for t in range(TT):
                    sl = slice(t * 512, (t + 1) * 512)
                    nc.vector.tensor_copy(xcmb[t][:], xa_ps[t][0:R, :])
                    nc.vector.tensor_tensor(
                        xcmb[t][:], xa_ps[t][32:32 + R, :], xcmb[t][:], ADD
                    )
                    nc.vector.tensor_tensor(
                        xcmb[t][:], xa_ps[t][64:64 + R, :], xcmb[t][:], ADD
                    )
                    nc.vector.tensor_tensor(
                        xa_sb[0:R, sl], xa_ps[t][96:96 + R, :], xcmb[t][:], ADD
                    )
                # duplicate for the row-tiled t=1 closes
                nc.vector.tensor_copy(xa_sb[32:32 + R, :], xa_sb[0:R, :])

            pts_of = {}
            ob_of = {}
            last_mains_mm = [None]

            def emit_mains(ot, fuse=None):
                # fuse: list of o-tiles emitted k-first (each x8 part feeds
                # all of them before the next part) -- used for the first
                # LAG tiles, which are paced by the x8 DMA stream
                ots = fuse if fuse is not None else [ot]
                for o in ots:
                    if o + WBUFS < OT:
                        issue_w(o + WBUFS, nc.sync)
                w_sbs = {o: w_tiles.pop(o) for o in ots}
                for o in ots:
                    pts_of[o] = [
                        psp.tile([128, 512], F32, tag="pt", name=f"pt{o}_{t}")
                        for t in range(TT)
                    ]
                if fuse is not None:
                    # HAM warmup: the first ~13us are paced by the x8 DMA
                    # stream, so the PE sits mostly idle and the clock gate
                    # keeps it at 1.2 GHz into the real work.  Junk matmuls
                    # on the W0 tile (lands ~3.5us) keep the activity monitor
                    # busy; the real c=0 start=True overwrites the psum.
                    w0 = w_sbs[ots[0]]
                    jp = pts_of[ots[0]][0]
                    for j in range(90):
                        nc.tensor.matmul(
                            jp[:, 0:128],
                            w0[:, j % KC2, :, :],
                            w0[:, (j + 1) % KC2, :, :],
                            start=True,
                            stop=True,
                            perf_mode=mybir.MatmulPerfMode.DoubleRow,
                        )
                for c in range(KC2):
                    for o in ots:
                        for t in range(TT):
                            mm = nc.tensor.matmul(
                                pts_of[o][t][:],
                                w_sbs[o][:, c, :, :],
                                x8_sl(c, t),
                                start=(c == 0),
                                stop=(o < CUT and c == KC2 - 1),
                                perf_mode=mybir.MatmulPerfMode.DoubleRow,
                            )
                last_mains_mm[0] = mm.ins
                for ot in ots:
                    emit_mains_drain(ot)

            def emit_mains_drain(ot):
                late = ot < CUT
                if late:
                    # drain mains-only result (unscaled, bf16) and free psum;
                    # the LoRA close for this tile happens later.  On VECTOR:
                    # the scalar queue is busy issuing input DMAs early on,
                    # and a delayed drain stalls the mains psum rotation.
                    ob = obp.tile([128, T], BF16, tag="ob", name=f"ob{ot}")
                    ob_of[ot] = ob
                    pts = pts_of.pop(ot)
                    for t in range(TT):
                        nc.vector.tensor_copy(ob[:, t * 512:(t + 1) * 512], pts[t][:])

            def emit_folded_close(ot):
                pts = pts_of.pop(ot)
                y_sb = yp.tile([128, T], BF16, tag="y", name=f"y{ot}")
                for t in range(TT):
                    # t=0 at PE rows 0..16, t=1 at rows 32..48: the two
                    # rank-17 closes run concurrently (row tiling)
                    p0 = 32 * t
                    nc.tensor.matmul(
                        pts[t][:],
                        btb_sb[p0:p0 + R + 1, ot * 128:(ot + 1) * 128],
                        xa_sb[p0:p0 + R + 1, t * 512:(t + 1) * 512],
                        start=False,
                        stop=True,
                        tile_position=(p0, 0),
                    )
                for t in range(TT):
                    sl = slice(t * 512, (t + 1) * 512)
                    if t % 2 == 0:
                        nc.scalar.activation(
                            y_sb[:, sl],
                            pts[t][:],
                            mybir.ActivationFunctionType.Identity,
                            scale=INV_S,
                        )
                    else:
                        nc.vector.tensor_scalar_mul(y_sb[:, sl], pts[t][:], INV_S)
                    if ot == OT - 1:
                        # per-half DMA on the last tile (t1 via sync, which is
                        # idle by now) so issue+transfer overlap the tail
                        eng = nc.scalar if t == 0 else nc.sync
                        eng.dma_start(y_out[ot][:, sl], y_sb[:, sl])
                if ot != OT - 1:
                    nc.scalar.dma_start(y_out[ot], y_sb[:])

            def emit_late_close(ot):
                ob = ob_of.pop(ot)
                y_sb = yp.tile([128, T], BF16, tag="y", name=f"y{ot}")
                for t in range(TT):
                    sl = slice(t * 512, (t + 1) * 512)
                    p0 = 32 * t
                    lps = psx.tile([128, 512], F32, tag="px", name=f"lps{ot}_{t}")
                    mm = nc.tensor.matmul(
                        lps[:],
                        btbl_sb[p0:p0 + R + 1, ot * 128:(ot + 1) * 128],
                        xa_sb[p0:p0 + R + 1, t * 512:(t + 1) * 512],
                        start=True,
                        stop=True,
                        tile_position=(p0, 0),
                    )
                    if t == 0 and last_mains_mm[0] is not None:
                        add_dep_helper(
                            mm.ins, last_mains_mm[0], reason="close after mains"
                        )
                    # y = (mains * INV_S) + (lora + bias)
                    nc.vector.scalar_tensor_tensor(
                        y_sb[:, sl], ob[:, sl], INV_S, lps[:], MULT, ADD
                    )
                nc.scalar.dma_start(y_out[ot], y_sb[:])

            # --- schedule
            emit_mains(None, fuse=list(range(LAG)))
            for ot in range(LAG, OT):
                r = ot - XA_R0
                if 0 <= r < KC // 4:
                    emit_xa_round(r)
                if ot == XA_R0 + KC // 4:
                    emit_xa_combine()
                q = ot - LC0
                if 0 <= q < CUT:
                    emit_late_close(q)
                emit_mains(ot)
                if ot >= CUT:
                    emit_folded_close(ot)

    return nc


def kernel(x, W, b, A, B):
    global LAST_RESULT
    x = np.ascontiguousarray(x, dtype=np.float32)
    W = np.ascontiguousarray(W, dtype=np.float32)

    # host quantization + layout prep (k lands on SBUF partitions; every DMA
    # is one fully-contiguous transfer)
    x8 = np.clip(x * SX, -240.0, 240.0).astype(FP8_NP)
    x8_dev = np.ascontiguousarray(
        x8.reshape(N_CORES, T, KC2, 2, 128).transpose(0, 4, 2, 3, 1)
    )  # [core, p, kpair, plane, t]
    xb_dev = np.ascontiguousarray(
        x.astype(ml_dtypes.bfloat16).reshape(N_CORES, T, KC, 128).transpose(0, 3, 2, 1)
    )  # [core, p, kc, t]
    w8 = np.clip(W * SW, -240.0, 240.0).astype(FP8_NP)
    w8_dev = np.ascontiguousarray(
        w8.reshape(OT, 128, KC2, 2, 128).transpose(0, 4, 2, 3, 1)
    )  # [ot, p, kpair, plane, o]
    at_dev = np.ascontiguousarray(
        np.asarray(A, dtype=np.float32).T.reshape(KC, 128, R).transpose(1, 0, 2)
    ).astype(ml_dtypes.bfloat16)  # [p, kc, r]
    btb_f32 = np.concatenate(
        [
            (SCALING * SX * SW) * np.asarray(B, dtype=np.float32).T,
            (SX * SW) * np.asarray(b, dtype=np.float32)[None, :],
        ],
        axis=0,
    )  # [17, O]

    def dup49(m):  # duplicate rank-17 stationary at partition offset 32
        out = np.zeros((49, O), dtype=np.float32)
        out[0:R + 1] = m
        out[32:32 + R + 1] = m
        return np.ascontiguousarray(out).astype(ml_dtypes.bfloat16)

    btb_dev = dup49(btb_f32)
    btbl_dev = dup49(btb_f32 * INV_S)

    nc = _build_kernel()
    in_maps = [
        {
            "x8": x8_dev[c],
            "xb": xb_dev[c],
            "w8": w8_dev,
            "at": at_dev,
            "btb": btb_dev,
            "btbl": btbl_dev,
        }
        for c in range(N_CORES)
    ]
    res = run_bass_kernel_spmd(nc, in_maps, list(range(N_CORES)))
    LAST_RESULT = res

    out = np.stack(
        [np.asarray(res.results[c]["y"], dtype=np.float32) for c in range(N_CORES)]
    )  # [c, ot, o, t]
    return np.ascontiguousarray(
        out.transpose(0, 3, 1, 2).reshape(NTOK, O)
    )
